# revision 8
# baseline (speedup 1.0000x reference)
"""Trainium2 Bass kernel for nn_MiddleFusionModule.

out = LayerNorm(node + sigmoid(node@Wg1 + (t@Wg2+bg)[seg]) * t[seg]),
t = relu(text@W1+b1)@W2+b2, over 131072 nodes sharded across 8 cores.

Strategy (one SPMD program, 8 data-parallel cores):
 - Host passes node_feat TRANSPOSED (feature-major [256, N]) so the big
   matmul needs no on-chip input transpose, plus a one-hot segment
   matrix [64, N] so the per-node text gather becomes two tiny-K
   matmuls (uniform across cores).
 - All matmuls run as float32r (full-rate fp32, ~1e-4 rel err).
 - Feature-major epilogue: sigmoid on ACT, gate*t_node on DVE,
   +node on GPSIMD, then PE transpose-mode flips 128x128 blocks to
   node-major PSUM where LayerNorm runs (bn_stats + Newton rsqrt +
   ACT affine).
"""

import sys

sys.path.insert(0, "/opt/trn_rl_repo")

from contextlib import ExitStack

import numpy as np

import concourse.bacc as bacc
import concourse.mybir as mybir
import concourse.tile as tile
from concourse.bass_utils import run_bass_kernel_spmd
from concourse.masks import make_identity

F32 = mybir.dt.float32
F32R = mybir.dt.float32r
AF = mybir.ActivationFunctionType
N_CORES = 8
D = 256          # node dim
TD = 768         # text dim
HD = 1024        # hidden dim
B = 64           # batch (segments)
CHUNK = 512      # nodes per inner chunk
LN_EPS = 1e-3


def _build(npc: int, apply_gb: bool):
    """Build the single SPMD program for `npc` nodes per core."""
    nch = npc // CHUNK
    nc = bacc.Bacc("TRN2", target_bir_lowering=False, debug=False,
                   num_devices=N_CORES)

    nodeT = nc.dram_tensor("nodeT", [D, npc], F32, kind="ExternalInput")
    onehot = nc.dram_tensor("onehot", [B, npc], F32, kind="ExternalInput")
    textT = nc.dram_tensor("textT", [TD, B], F32, kind="ExternalInput")
    w1 = nc.dram_tensor("w1", [TD, HD], F32, kind="ExternalInput")
    b1 = nc.dram_tensor("b1", [1, HD], F32, kind="ExternalInput")
    w2 = nc.dram_tensor("w2", [HD, D], F32, kind="ExternalInput")
    b2 = nc.dram_tensor("b2", [1, D], F32, kind="ExternalInput")
    wg1 = nc.dram_tensor("wg1", [D, D], F32, kind="ExternalInput")
    wg2 = nc.dram_tensor("wg2", [D, D], F32, kind="ExternalInput")
    bg = nc.dram_tensor("bg", [1, D], F32, kind="ExternalInput")
    gamma = nc.dram_tensor("gamma", [1, D], F32, kind="ExternalInput")
    beta = nc.dram_tensor("beta", [1, D], F32, kind="ExternalInput")
    onesd = nc.dram_tensor("onesd", [1, B], F32, kind="ExternalInput")
    out = nc.dram_tensor("out", [npc, D], F32, kind="ExternalOutput")

    with tile.TileContext(nc) as tc:
        with ExitStack() as ctx:
            consts = ctx.enter_context(tc.tile_pool(name="consts", bufs=1))

            # ---- constants / weights in SBUF ----
            tx_sb = consts.tile([128, 6, B], F32R)
            nc.sync.dma_start(out=tx_sb, in_=textT.bitcast(F32R).rearrange("(c k) m -> k c m", c=6))
            w1_sb = consts.tile([128, 6, HD], F32R)
            nc.sync.dma_start(out=w1_sb, in_=w1.bitcast(F32R).rearrange("(c k) n -> k c n", c=6))
            w2_sb = consts.tile([128, 8, D], F32R)
            nc.sync.dma_start(out=w2_sb, in_=w2.bitcast(F32R).rearrange("(c k) n -> k c n", c=8))
            wg1_sb = consts.tile([128, 2, D], F32R)
            nc.sync.dma_start(out=wg1_sb, in_=wg1.bitcast(F32R).rearrange("(c k) n -> k c n", c=2))
            wg2_sb = consts.tile([128, 2, D], F32R)
            nc.sync.dma_start(out=wg2_sb, in_=wg2.bitcast(F32R).rearrange("(c k) n -> k c n", c=2))
            b1_sb = consts.tile([1, HD], F32R)
            nc.sync.dma_start(out=b1_sb, in_=b1.bitcast(F32R)[:, :])
            b2_sb = consts.tile([1, D], F32R)
            nc.sync.dma_start(out=b2_sb, in_=b2.bitcast(F32R)[:, :])
            bg_sb = consts.tile([1, D], F32R)
            nc.sync.dma_start(out=bg_sb, in_=bg.bitcast(F32R)[:, :])
            ones64 = consts.tile([1, B], F32R)
            nc.sync.dma_start(out=ones64, in_=onesd.bitcast(F32R)[:, :])
            ident = consts.tile([128, 128], F32)
            make_identity(nc, ident)
            t_sb = consts.tile([B, D], F32R)    # text rows, node-dim
            u_sb = consts.tile([B, D], F32R)    # (t @ Wg2 + bg) rows

            def R(ap):
                return ap.bitcast(F32R)

            # ---- text MLP (one-time, tiny) ----
            with ExitStack() as mctx:
                mp = mctx.enter_context(tc.tile_pool(name="mlp", bufs=1))
                mps = mctx.enter_context(
                    tc.tile_pool(name="mlp_ps", bufs=1, space="PSUM"))
                ps_t1 = mps.tile([B, 2, 512], F32)
                for h in range(2):
                    for k in range(6):
                        nc.tensor.matmul(
                            ps_t1[:, h, :], R(tx_sb[:, k, :]),
                            R(w1_sb[:, k, h * 512:(h + 1) * 512]),
                            start=(k == 0), stop=False)
                    nc.tensor.matmul(
                        ps_t1[:, h, :], R(ones64),
                        R(b1_sb[:, h * 512:(h + 1) * 512]),
                        start=False, stop=True)
                t1_sb = mp.tile([B, 2, 512], F32)
                for h in range(2):
                    nc.scalar.activation(out=t1_sb[:, h, :], in_=ps_t1[:, h, :],
                                         func=AF.Relu)
                # transpose t1 -> t1T [1024, 64] as [128, 8, 64]
                t1T_sb = mp.tile([128, 8, B], F32R)
                ps_tr = mps.tile([128, B], F32)
                for j in range(8):
                    src = t1_sb[:, j // 4, (j % 4) * 128:(j % 4 + 1) * 128]
                    nc.tensor.matmul(ps_tr, src, ident[:B, :B],
                                     is_transpose=True, start=True, stop=True)
                    nc.vector.tensor_copy(out=t1T_sb[:, j, :], in_=ps_tr)
                ps_t = mps.tile([B, D], F32)
                for j in range(8):
                    nc.tensor.matmul(ps_t, R(t1T_sb[:, j, :]), R(w2_sb[:, j, :]),
                                     start=(j == 0), stop=False)
                nc.tensor.matmul(ps_t, R(ones64), R(b2_sb), start=False, stop=True)
                nc.vector.tensor_copy(out=t_sb, in_=ps_t)
                # transpose t -> tT [256, 64] as [128, 2, 64]
                tT_sb = mp.tile([128, 2, B], F32R)
                for c in range(2):
                    nc.tensor.matmul(ps_tr, t_sb[:, c * 128:(c + 1) * 128].bitcast(F32),
                                     ident[:B, :B],
                                     is_transpose=True, start=True, stop=True)
                    nc.vector.tensor_copy(out=tT_sb[:, c, :], in_=ps_tr)
                ps_u = mps.tile([B, D], F32)
                for c in range(2):
                    nc.tensor.matmul(ps_u, R(tT_sb[:, c, :]), R(wg2_sb[:, c, :]),
                                     start=(c == 0), stop=False)
                nc.tensor.matmul(ps_u, R(ones64), R(bg_sb), start=False, stop=True)
                nc.vector.tensor_copy(out=u_sb, in_=ps_u)

            # ---- main loop ----
            inp = ctx.enter_context(tc.tile_pool(name="inp", bufs=3))
            work = ctx.enter_context(tc.tile_pool(name="work", bufs=2))
            pz = ctx.enter_context(tc.tile_pool(name="pz", bufs=1, space="PSUM"))

            nodeTv = nodeT.bitcast(F32R).rearrange("(c k) n -> k c n", c=2)
            outv = out.rearrange("(ch j p) f -> ch p j f", p=128, j=4)

            gb_sb = None
            if apply_gb:
                gb_sb = consts.tile([128, 2, D], F32)
                for name, src, slot in (("g", gamma, 0), ("b", beta, 1)):
                    import concourse.bass as bass
                    bcast = bass.AP(tensor=src.ap().tensor, offset=0,
                                    ap=[[0, 128], [1, D]])
                    nc.gpsimd.dma_start(out=gb_sb[:, slot, :], in_=bcast)

            for ch in range(nch):
                sl = slice(ch * CHUNK, (ch + 1) * CHUNK)
                node_sb = inp.tile([128, 2, CHUNK], F32R, tag="node")
                nc.sync.dma_start(out=node_sb, in_=nodeTv[:, :, sl])
                oh_sb = inp.tile([B, CHUNK], F32R, tag="oh")
                nc.sync.dma_start(out=oh_sb, in_=onehot.bitcast(F32R)[:, sl])

                # z = Wg1^T-major preact, t_node rows (both feature-major)
                ps_z = pz.tile([128, 2, CHUNK], F32, tag="ps_z")
                ps_tn = pz.tile([128, 2, CHUNK], F32, tag="ps_tn")
                for c in range(2):
                    for k in range(2):
                        nc.tensor.matmul(
                            ps_z[:, c, :],
                            R(wg1_sb[:, k, c * 128:(c + 1) * 128]),
                            R(node_sb[:, k, :]),
                            start=(k == 0), stop=False)
                    nc.tensor.matmul(
                        ps_z[:, c, :], R(u_sb[:, c * 128:(c + 1) * 128]),
                        R(oh_sb), start=False, stop=True)
                    nc.tensor.matmul(
                        ps_tn[:, c, :], R(t_sb[:, c * 128:(c + 1) * 128]),
                        R(oh_sb), start=True, stop=True)

                gate_sb = work.tile([128, 2, CHUNK], F32, tag="gate")
                gt_sb = work.tile([128, 2, CHUNK], F32, tag="gt")
                enh_sb = work.tile([128, 2, CHUNK], F32, tag="enh")
                for c in range(2):
                    nc.scalar.activation(out=gate_sb[:, c, :], in_=ps_z[:, c, :],
                                         func=AF.Sigmoid)
                    nc.vector.tensor_mul(out=gt_sb[:, c, :],
                                         in0=gate_sb[:, c, :], in1=ps_tn[:, c, :])
                    nc.gpsimd.tensor_add(out=enh_sb[:, c, :],
                                         in0=gt_sb[:, c, :], in1=node_sb[:, c, :].bitcast(F32))

                # transpose to node-major PSUM: 4 node-tiles of [128, 256]
                ps_e = pz.tile([128, 2, CHUNK], F32, tag="ps_e")
                for j in range(4):
                    for c in range(2):
                        nc.tensor.matmul(
                            ps_e[:, j // 2, (j % 2) * 256 + c * 128:
                                 (j % 2) * 256 + (c + 1) * 128],
                            enh_sb[:, c, j * 128:(j + 1) * 128],
                            ident, is_transpose=True,
                            start=True, stop=True, skip_group_check=True)

                # LayerNorm over node-major psum
                st_sb = work.tile([128, 2, 2, 6], F32, tag="st")
                mv_sb = work.tile([128, 2, 2, 2], F32, tag="mv")
                for b in range(2):
                    for g in range(2):
                        nc.vector.bn_stats(
                            out=st_sb[:, b, g, :],
                            in_=ps_e[:, b, g * 256:(g + 1) * 256])
                        nc.vector.bn_aggr(out=mv_sb[:, b, g, :],
                                          in_=st_sb[:, b, g:g + 1, :])
                # rstd = 1/sqrt(var+eps) via recip-seeded Newton (2 iters)
                ve = work.tile([128, 2, 2, 1], F32, tag="ve")
                y = work.tile([128, 2, 2, 1], F32, tag="y")
                tmp = work.tile([128, 2, 2, 1], F32, tag="tmp")
                negms = work.tile([128, 2, 2, 1], F32, tag="negms")
                nc.vector.tensor_scalar_add(out=ve, in0=mv_sb[:, :, :, 1:2],
                                            scalar1=LN_EPS)
                nc.vector.reciprocal(out=y, in_=ve)
                nc.vector.tensor_scalar(out=y, in0=y, scalar1=0.5, scalar2=0.5,
                                        op0=mybir.AluOpType.mult,
                                        op1=mybir.AluOpType.add)
                for _ in range(2):
                    nc.vector.tensor_mul(out=tmp, in0=y, in1=y)
                    nc.vector.tensor_mul(out=tmp, in0=tmp, in1=ve)
                    nc.vector.tensor_scalar(out=tmp, in0=tmp, scalar1=-0.5,
                                            scalar2=1.5,
                                            op0=mybir.AluOpType.mult,
                                            op1=mybir.AluOpType.add)
                    nc.vector.tensor_mul(out=y, in0=y, in1=tmp)
                nc.vector.tensor_mul(out=negms, in0=mv_sb[:, :, :, 0:1], in1=y)
                nc.vector.tensor_scalar_mul(out=negms, in0=negms, scalar1=-1.0)

                out_sb = work.tile([128, 4, D], F32, tag="out")
                for b in range(2):
                    for g in range(2):
                        j = 2 * b + g
                        nc.scalar.activation(
                            out=out_sb[:, j, :],
                            in_=ps_e[:, b, g * 256:(g + 1) * 256],
                            func=AF.Identity,
                            bias=negms[:, b, g, :], scale=y[:, b, g, :])
                if apply_gb:
                    for j in range(4):
                        nc.vector.tensor_mul(out=out_sb[:, j, :],
                                             in0=out_sb[:, j, :],
                                             in1=gb_sb[:, 0, :])
                        nc.vector.tensor_add(out=out_sb[:, j, :],
                                             in0=out_sb[:, j, :],
                                             in1=gb_sb[:, 1, :])
                nc.sync.dma_start(out=outv[ch], in_=out_sb)

    nc.compile()
    return nc


_NC_CACHE = {}


def kernel(node_feat, text_feat, segment_ids, W1, b1, W2, b2, Wg, bg,
           ln_gamma, ln_beta):
    total, d = node_feat.shape
    npc = total // N_CORES
    assert npc % CHUNK == 0

    node_feat = np.asarray(node_feat, dtype=np.float32)
    nodeT = np.ascontiguousarray(node_feat.T)               # [256, total]
    textT = np.ascontiguousarray(np.asarray(text_feat, np.float32).T)
    seg = np.asarray(segment_ids)
    onehot = (seg[None, :] == np.arange(B, dtype=seg.dtype)[:, None]
              ).astype(np.float32)                          # [64, total]

    apply_gb = not (np.all(np.asarray(ln_gamma) == 1.0)
                    and np.all(np.asarray(ln_beta) == 0.0))

    key = (npc, apply_gb)
    if key not in _NC_CACHE:
        _NC_CACHE[key] = _build(npc, apply_gb)
    nc = _NC_CACHE[key]

    shared = {
        "textT": textT,
        "w1": np.asarray(W1, np.float32),
        "b1": np.asarray(b1, np.float32).reshape(1, HD),
        "w2": np.asarray(W2, np.float32),
        "b2": np.asarray(b2, np.float32).reshape(1, D),
        "wg1": np.ascontiguousarray(np.asarray(Wg, np.float32)[:D]),
        "wg2": np.ascontiguousarray(np.asarray(Wg, np.float32)[D:]),
        "bg": np.asarray(bg, np.float32).reshape(1, D),
        "gamma": np.asarray(ln_gamma, np.float32).reshape(1, D),
        "beta": np.asarray(ln_beta, np.float32).reshape(1, D),
        "onesd": np.ones((1, B), np.float32),
    }
    in_maps = []
    for c in range(N_CORES):
        m = dict(shared)
        m["nodeT"] = np.ascontiguousarray(nodeT[:, c * npc:(c + 1) * npc])
        m["onehot"] = np.ascontiguousarray(onehot[:, c * npc:(c + 1) * npc])
        in_maps.append(m)

    res = run_bass_kernel_spmd(nc, in_maps, core_ids=list(range(N_CORES)))
    out = np.concatenate([res.results[c]["out"] for c in range(N_CORES)], axis=0)
    return out.astype(np.float32)


def bench_device(inputs, iters=6):
    """Time repeated on-device executions (8 cores, inputs device-resident).

    Returns median seconds per execution (max over cores, incl. PJRT
    dispatch overhead of ~1ms)."""
    import time

    import jax
    import jax.numpy as jnp
    from jax.experimental.shard_map import shard_map
    from jax.sharding import Mesh, PartitionSpec

    import concourse.bass2jax as b2j
    import concourse.mybir as mb

    node_feat = np.asarray(inputs["node_feat"], np.float32)
    total = node_feat.shape[0]
    npc = total // N_CORES
    seg = np.asarray(inputs["segment_ids"])
    nodeT = np.ascontiguousarray(node_feat.T)
    onehot = (seg[None, :] == np.arange(B, dtype=seg.dtype)[:, None]
              ).astype(np.float32)
    key = (npc, False)
    if key not in _NC_CACHE:
        _NC_CACHE[key] = _build(npc, False)
    nc = _NC_CACHE[key]
    shared = {
        "textT": np.ascontiguousarray(np.asarray(inputs["text_feat"], np.float32).T),
        "w1": np.asarray(inputs["W1"], np.float32),
        "b1": np.asarray(inputs["b1"], np.float32).reshape(1, HD),
        "w2": np.asarray(inputs["W2"], np.float32),
        "b2": np.asarray(inputs["b2"], np.float32).reshape(1, D),
        "wg1": np.ascontiguousarray(np.asarray(inputs["Wg"], np.float32)[:D]),
        "wg2": np.ascontiguousarray(np.asarray(inputs["Wg"], np.float32)[D:]),
        "bg": np.asarray(inputs["bg"], np.float32).reshape(1, D),
        "gamma": np.asarray(inputs["ln_gamma"], np.float32).reshape(1, D),
        "beta": np.asarray(inputs["ln_beta"], np.float32).reshape(1, D),
        "onesd": np.ones((1, B), np.float32),
    }
    in_maps = []
    for c in range(N_CORES):
        m = dict(shared)
        m["nodeT"] = np.ascontiguousarray(nodeT[:, c * npc:(c + 1) * npc])
        m["onehot"] = np.ascontiguousarray(onehot[:, c * npc:(c + 1) * npc])
        in_maps.append(m)

    b2j.install_neuronx_cc_hook()
    partition_name = (nc.partition_id_tensor.name
                      if nc.partition_id_tensor else None)
    in_names, out_names, out_avals, zero_outs = [], [], [], []
    for alloc in nc.m.functions[0].allocations:
        if not isinstance(alloc, mb.MemoryLocationSet):
            continue
        name = alloc.memorylocations[0].name
        if alloc.kind == "ExternalInput":
            if name != partition_name:
                in_names.append(name)
        elif alloc.kind == "ExternalOutput":
            out_names.append(name)
            shape = tuple(alloc.tensor_shape)
            dtype = mb.dt.np(alloc.dtype)
            out_avals.append(jax.core.ShapedArray(shape, dtype))
            zero_outs.append(np.zeros(shape, dtype))
    n_params = len(in_names)
    n_outs = len(out_avals)
    in_names_all = list(in_names) + out_names
    if partition_name is not None:
        in_names_all.append(partition_name)
    donate = tuple(range(n_params, n_params + n_outs))

    def _body(*args):
        operands = list(args)
        if partition_name is not None:
            operands.append(b2j.partition_id_tensor())
        outs = b2j._bass_exec_p.bind(
            *operands, out_avals=tuple(out_avals), in_names=tuple(in_names_all),
            out_names=tuple(out_names), lowering_input_output_aliases=(),
            sim_require_finite=True, sim_require_nnan=True, nc=nc)
        return tuple(outs)

    devices = jax.devices()[:N_CORES]
    mesh = Mesh(np.asarray(devices), ("core",))
    sharded = jax.jit(
        shard_map(_body, mesh=mesh,
                  in_specs=(PartitionSpec("core"),) * (n_params + n_outs),
                  out_specs=(PartitionSpec("core"),) * n_outs,
                  check_rep=False),
        donate_argnums=donate, keep_unused=True)
    concat_in = [
        np.concatenate([np.asarray(in_maps[c][nm]) for c in range(N_CORES)], axis=0)
        for nm in in_names]
    sh = jax.sharding.NamedSharding(mesh, PartitionSpec("core"))
    in_dev = [jax.device_put(a, sh) for a in concat_in]
    times = []
    for it in range(iters):
        zs = [jax.device_put(
            np.zeros((N_CORES * z.shape[0], *z.shape[1:]), z.dtype), sh)
            for z in zero_outs]
        jax.block_until_ready(zs)
        t0 = time.perf_counter()
        outs = sharded(*in_dev, *zs)
        jax.block_until_ready(outs)
        times.append(time.perf_counter() - t0)
    times.sort()
    return times[len(times) // 2], times


def run_traced(inputs):
    """Re-run with NTFF tracing; returns max-core exec time in ns (or None)."""
    global _LAST_TRACE
    import kernel as K  # ensure cache shared

    node_feat = np.asarray(inputs["node_feat"], np.float32)
    total = node_feat.shape[0]
    npc = total // N_CORES
    seg = np.asarray(inputs["segment_ids"])
    nodeT = np.ascontiguousarray(node_feat.T)
    onehot = (seg[None, :] == np.arange(B, dtype=seg.dtype)[:, None]
              ).astype(np.float32)
    apply_gb = not (np.all(np.asarray(inputs["ln_gamma"]) == 1.0)
                    and np.all(np.asarray(inputs["ln_beta"]) == 0.0))
    key = (npc, apply_gb)
    if key not in _NC_CACHE:
        _NC_CACHE[key] = _build(npc, apply_gb)
    nc = _NC_CACHE[key]
    shared = {
        "textT": np.ascontiguousarray(np.asarray(inputs["text_feat"], np.float32).T),
        "w1": np.asarray(inputs["W1"], np.float32),
        "b1": np.asarray(inputs["b1"], np.float32).reshape(1, HD),
        "w2": np.asarray(inputs["W2"], np.float32),
        "b2": np.asarray(inputs["b2"], np.float32).reshape(1, D),
        "wg1": np.ascontiguousarray(np.asarray(inputs["Wg"], np.float32)[:D]),
        "wg2": np.ascontiguousarray(np.asarray(inputs["Wg"], np.float32)[D:]),
        "bg": np.asarray(inputs["bg"], np.float32).reshape(1, D),
        "gamma": np.asarray(inputs["ln_gamma"], np.float32).reshape(1, D),
        "beta": np.asarray(inputs["ln_beta"], np.float32).reshape(1, D),
        "onesd": np.ones((1, B), np.float32),
    }
    in_maps = []
    for c in range(N_CORES):
        m = dict(shared)
        m["nodeT"] = np.ascontiguousarray(nodeT[:, c * npc:(c + 1) * npc])
        m["onehot"] = np.ascontiguousarray(onehot[:, c * npc:(c + 1) * npc])
        in_maps.append(m)
    res = run_bass_kernel_spmd(nc, in_maps, core_ids=list(range(N_CORES)),
                               trace=True)
    _LAST_TRACE = res
    return res.exec_time_ns
